# revision 9
# baseline (speedup 1.0000x reference)
"""Causal self-attention (T=2048, C=1024, H=16) on 8 trn2 NeuronCores.

Tensor-parallel over heads: core i computes heads 2i, 2i+1 (q/k/v rows
128i:128i+128 of each 1024-row block of wqkv_w, proj_w columns
128i:128i+128), producing a partial output projection; partials are summed
on the host (the all-reduce of the sharding hint).

Per-core Bass/Tile kernel (matmuls in float32r = full-rate fp32):
  B. qkvT[j, t] = wqkv_aug.T @ xT_aug   (bias via ones-row, q pre-scaled 1/32)
  C. vT -> v_aug[k, 65] tiles via PE transpose, ones column appended so the
     PV matmul's 65th output row is the softmax denominator.
  D. per (head, 512-col t-chunk): sT[k, t] = kT.T @ qT per k-tile ->
     exp on ScalarE -> causal mask via gpsimd affine_select (diagonal tiles
     only; fully-masked k-tiles skipped) -> attnT[d, t] += v_aug.T @ exp_w.
     Normalize: reciprocal of sums row, partition-broadcast via K=1 matmul
     with ones, DVE multiply.
  E. partialT[o, t] = projT.T @ attn (two K=64 matmuls, one per head).
"""

import sys

if "/opt/trn_rl_repo" not in sys.path:
    sys.path.insert(0, "/opt/trn_rl_repo")

import numpy as np

T = 2048
C = 1024
CH = 512  # t-chunk width (one PSUM bank of fp32)
NT = T // CH  # 4 t-chunks
NK = T // 128  # 16 k-tiles
NCT = C // 128  # 8 contraction tiles (+1 ones/bias row)
N_CORES = 8

_CACHE = {}


def _build():
    import concourse.tile as tile
    from concourse import bacc, mybir

    F32 = mybir.dt.float32
    F32R = mybir.dt.float32r
    EXP = mybir.ActivationFunctionType.Exp
    IS_GE = mybir.AluOpType.is_ge

    def r(ap):
        return ap.bitcast(F32R)

    nc = bacc.Bacc(
        "TRN2",
        target_bir_lowering=False,
        debug=False,
        enable_asserts=False,
        num_devices=N_CORES,
    )
    xT = nc.dram_tensor("xT", [C + 1, T], F32R, kind="ExternalInput").ap()
    wqkv = nc.dram_tensor("wqkv", [C + 1, 384], F32R, kind="ExternalInput").ap()
    projT = nc.dram_tensor("projT", [128, C], F32R, kind="ExternalInput").ap()
    consts = nc.dram_tensor("consts", [128, 192], F32R, kind="ExternalInput").ap()
    out = nc.dram_tensor("out", [C, T], F32, kind="ExternalOutput").ap()

    with tile.TileContext(nc) as tc:
        with (
            tc.tile_pool(name="big", bufs=1) as big,
            tc.tile_pool(name="expw", bufs=4) as expw_pool,
            tc.tile_pool(name="attn_tmp", bufs=2) as attn_tmp_pool,
            tc.tile_pool(name="outev", bufs=3) as outev_pool,
            tc.tile_pool(name="ps", bufs=1, space="PSUM") as ps,
        ):
            # ---- resident SBUF tensors -------------------------------------
            x_sb = big.tile([128, NCT + 1, T], F32R, name="x_sb")
            w_sb = big.tile([128, NCT + 1, 384], F32R, name="w_sb")
            proj0_sb = big.tile([64, C], F32R, name="proj0_sb")
            proj1_sb = big.tile([64, C], F32R, name="proj1_sb")
            qT_sb = big.tile([128, T], F32R, name="qT_sb")
            kT_sb = big.tile([128, T], F32R, name="kT_sb")
            vT_sb = big.tile([128, T], F32R, name="vT_sb")
            v_aug0 = big.tile([128, NK, 65], F32R, name="v_aug0")
            v_aug1 = big.tile([128, NK, 65], F32R, name="v_aug1")
            attn0 = big.tile([64, T], F32R, name="attn0")
            attn1 = big.tile([64, T], F32R, name="attn1")
            # consts: cols 0:64 = two stacked 64x64 identities, cols 64:192 = ones
            consts_sb = big.tile([128, 192], F32R, name="consts_sb")

            for ct in range(NCT):
                nc.sync.dma_start(
                    out=x_sb[:, ct, :], in_=xT[128 * ct : 128 * ct + 128, :]
                )
                nc.sync.dma_start(
                    out=w_sb[:, ct, :], in_=wqkv[128 * ct : 128 * ct + 128, :]
                )
            nc.sync.dma_start(out=x_sb[0:1, NCT, :], in_=xT[C : C + 1, :])
            nc.sync.dma_start(out=w_sb[0:1, NCT, :], in_=wqkv[C : C + 1, :])
            nc.sync.dma_start(out=proj0_sb, in_=projT[0:64, :])
            nc.sync.dma_start(out=proj1_sb, in_=projT[64:128, :])
            nc.sync.dma_start(out=consts_sb, in_=consts)
            ident = consts_sb[:, 0:64]

            ones16 = consts[:, 64:80].rearrange("p (a b) -> p a b", b=1)
            nc.sync.dma_start(out=v_aug0[:, :, 64:65], in_=ones16)
            nc.sync.dma_start(out=v_aug1[:, :, 64:65], in_=ones16)

            # ---- stage B: QKV ---------------------------------------------
            for part, dest in ((0, qT_sb), (1, kT_sb), (2, vT_sb)):
                cols = slice(128 * part, 128 * part + 128)
                for c in range(NT):
                    tcol = slice(CH * c, CH * c + CH)
                    qkv_ps = ps.tile(
                        [128, CH], F32, tag="m", bufs=3, name=f"qkvps_{part}_{c}"
                    )
                    for ct in range(NCT + 1):
                        kp = 128 if ct < NCT else 1
                        nc.tensor.matmul(
                            qkv_ps,
                            w_sb[0:kp, ct, cols],
                            x_sb[0:kp, ct, tcol],
                            start=(ct == 0),
                            stop=(ct == NCT),
                        )
                    nc.vector.tensor_copy(dest[:, tcol], qkv_ps)

            # ---- stage C: v transpose + augment ---------------------------
            for h, v_aug in ((0, v_aug0), (1, v_aug1)):
                hrow = slice(64 * h, 64 * h + 64)
                for kt in range(NK):
                    tr_ps = ps.tile([128, 64], F32, tag="m", bufs=3, name=f"tr_{h}_{kt}")
                    nc.tensor.transpose(
                        r(tr_ps),
                        vT_sb[hrow, 128 * kt : 128 * kt + 128],
                        ident[hrow, :],
                    )
                    nc.vector.tensor_copy(v_aug[:, kt, 0:64], tr_ps)

            # ---- stages D+E per t-chunk -----------------------------------
            for c in range(NT):
                tcol = slice(CH * c, CH * c + CH)
                nj = 4 * c + 4
                for h, v_aug, attn in ((0, v_aug0, attn0), (1, v_aug1, attn1)):
                    hrow = slice(64 * h, 64 * h + 64)
                    pv_ps = ps.tile([65, CH], F32, tag="pv", bufs=2, name=f"pv_{h}_{c}")
                    for j in range(nj):
                        s_ps = ps.tile(
                            [128, CH], F32, tag="s", bufs=3, name=f"s_{h}_{c}_{j}"
                        )
                        nc.tensor.matmul(
                            s_ps,
                            kT_sb[hrow, 128 * j : 128 * j + 128],
                            qT_sb[hrow, tcol],
                            start=True,
                            stop=True,
                        )
                        w_t = expw_pool.tile(
                            [128, CH], F32R, tag="expw", name=f"w_{h}_{c}_{j}"
                        )
                        nc.scalar.activation(out=w_t, in_=s_ps, func=EXP)
                        diag = j - 4 * c
                        if diag >= 0:
                            # keep exp(score) where t >= k, i.e. f - p - 128*diag >= 0
                            nc.gpsimd.affine_select(
                                out=w_t,
                                in_=w_t,
                                pattern=[[1, CH]],
                                compare_op=IS_GE,
                                fill=0.0,
                                base=-128 * diag,
                                channel_multiplier=-1,
                            )
                        nc.tensor.matmul(
                            pv_ps,
                            v_aug[:, j, :],
                            w_t,
                            start=(j == 0),
                            stop=(j == nj - 1),
                        )
                    at = attn_tmp_pool.tile(
                        [65, CH], F32R, tag="attn_tmp", name=f"at_{h}_{c}"
                    )
                    nc.vector.tensor_copy(at, pv_ps)
                    with nc.allow_low_precision(reason="fp32r matmul operand"):
                        nc.vector.reciprocal(at[64:65, :], at[64:65, :])
                    rb_ps = ps.tile([128, CH], F32, tag="m", bufs=3, name=f"rb_{h}_{c}")
                    nc.tensor.matmul(
                        rb_ps,
                        consts_sb[64:65, 64:192],
                        at[64:65, :],
                        start=True,
                        stop=True,
                    )
                    nc.vector.tensor_mul(attn[:, tcol], at[0:64, :], rb_ps[0:64, :])

                # ---- stage E: output projection for this chunk -------------
                for m in range(8):
                    pr_ps = ps.tile([128, CH], F32, tag="m", bufs=3, name=f"pr_{m}_{c}")
                    nc.tensor.matmul(
                        pr_ps,
                        proj0_sb[:, 128 * m : 128 * m + 128],
                        attn0[:, tcol],
                        start=True,
                        stop=False,
                    )
                    nc.tensor.matmul(
                        pr_ps,
                        proj1_sb[:, 128 * m : 128 * m + 128],
                        attn1[:, tcol],
                        start=False,
                        stop=True,
                    )
                    ob = outev_pool.tile([128, CH], F32, tag="outev", name=f"ob_{m}_{c}")
                    nc.vector.tensor_copy(ob, pr_ps)
                    nc.sync.dma_start(
                        out=out[128 * m : 128 * m + 128, tcol], in_=ob
                    )

    nc.compile()
    return nc


def _get_nc():
    if "nc" not in _CACHE:
        _CACHE["nc"] = _build()
    return _CACHE["nc"]


def _make_consts():
    consts = np.zeros((128, 192), np.float32)
    consts[0:64, 0:64] = np.eye(64, dtype=np.float32)
    consts[64:128, 0:64] = np.eye(64, dtype=np.float32)
    consts[:, 64:192] = 1.0
    return consts


def _make_in_maps(x, wqkv_w, wqkv_b, proj_w):
    xT = np.concatenate(
        [np.asarray(x, np.float32).T, np.ones((1, T), np.float32)], axis=0
    )
    xT = np.ascontiguousarray(xT)
    consts = _make_consts()
    scale = np.float32(1.0 / np.sqrt(C))
    in_maps = []
    for i in range(N_CORES):
        rows = []
        biases = []
        for blk, s in ((0, scale), (1, None), (2, None)):
            sl = slice(blk * C + 128 * i, blk * C + 128 * i + 128)
            w = np.asarray(wqkv_w[sl], np.float32)
            b = np.asarray(wqkv_b[sl], np.float32)
            if s is not None:
                w = w * s
                b = b * s
            rows.append(w)
            biases.append(b)
        W = np.concatenate(rows, axis=0)  # [384, 1024]
        B = np.concatenate(biases, axis=0)  # [384]
        wqkv_aug = np.concatenate([W, B[:, None]], axis=1).T  # [1025, 384]
        pT = np.asarray(proj_w[:, 128 * i : 128 * i + 128], np.float32).T  # [128, 1024]
        in_maps.append(
            {
                "xT": xT,
                "wqkv": np.ascontiguousarray(wqkv_aug),
                "projT": np.ascontiguousarray(pT),
                "consts": consts,
            }
        )
    return in_maps


def kernel(x, wqkv_w, wqkv_b, proj_w, proj_b, _trace=False, _tmpdir=None):
    from concourse.bass_utils import run_bass_kernel_spmd

    nc = _get_nc()
    in_maps = _make_in_maps(x, wqkv_w, wqkv_b, proj_w)
    res = run_bass_kernel_spmd(
        nc,
        in_maps,
        core_ids=list(range(N_CORES)),
        trace=_trace,
        tmpdir=_tmpdir,
    )
    acc = np.zeros((C, T), np.float64)
    for rmap in res.results:
        acc += rmap["out"].astype(np.float64)
    full = acc.T + np.asarray(proj_b, np.float64)[None, :]
    if _trace:
        _CACHE["last_result"] = res
    return full.astype(np.float32)
